# revision 3
# baseline (speedup 1.0000x reference)
"""DFlash draft-model kernel for 8 Trainium2 NeuronCores.

Sharding: data-parallel over B (cores 0-3 -> b=0, 4-7 -> b=1). Within a
group of 4 lanes: ctx-projection + K/V projection sharded by kv rows
(lane r: ctx rows [512r,512r+512) + draft rows [256r,256r+256)), K/V
AllGathered within the group, then attention + lm_head sharded by the
lane's 256 draft tokens (full vocab per core, so softmax stays local).
Loss/accuracy reduction of the per-core logits happens on host.
"""
import math

import ml_dtypes
import numpy as np
import orjson

import concourse.bass as bass
import concourse.bass_utils as _bu
import concourse.bass2jax as _b2j
import concourse.mybir as mybir
import concourse.tile as _tile
from concourse.bass_utils import run_bass_kernel_spmd
from concourse.tile import TileContext
from concourse.vector_clock import ScopedClock, VectorClock

# ---------------------------------------------------------------------------
# Workarounds: this walrus accepts max 1 sync-wait per instruction
# (2 for EventSemaphore).
# ---------------------------------------------------------------------------


def _split_drain_and_barrier(self, tick_clock, wait_clock):
    gc = tick_clock.global_clock
    n = len(gc)
    for p in range(n):
        t = gc[p]
        if t <= 0:
            continue
        vc = VectorClock([0] * n)
        vc.require_at_least(p, t)
        d = self.nc.sync.drain()
        wait_clock.add_sem_waits(d.ins, ScopedClock({None: vc}))
    self.nc.all_engine_barrier()
    assert self.sems is not None
    popped = self.nc._tile_sem_poison_stack.pop()
    assert popped is self._sem_poison
    self.nc.clear_and_free_semaphores(list(self.sems.allocated().values()))
    self.nc.all_engine_barrier()


_tile.TileContext._drain_and_barrier = _split_drain_and_barrier


def _split_multiwait_bir(bir_json: bytes) -> bytes:
    d = orjson.loads(bir_json)
    ctr = 0
    changed = False
    for fn in d.get("functions", []):
        for b in fn.get("blocks", []):
            out = []
            for i in b.get("instructions", []):
                si = i.get("sync_info")
                waits = (si or {}).get("on_wait") or []
                cap = 2 if i.get("opcode") == "EventSemaphore" else 1
                if len(waits) > cap:
                    changed = True
                    extra = waits[:-cap]
                    si["on_wait"] = waits[-cap:]
                    for j in range(0, len(extra), 2):
                        ctr += 1
                        out.append({
                            "debug": i.get("debug"),
                            "engine": i["engine"],
                            "ins": [],
                            "name": f"mwfix-{ctr}-{i['name']}",
                            "opcode": "EventSemaphore",
                            "outs": [],
                            "sync_info": {"on_update": [],
                                          "on_wait": extra[j:j + 2]},
                        })
                out.append(i)
            b["instructions"] = out
    return orjson.dumps(d) if changed else bir_json


_orig_compile_bir_kernel = _bu.compile_bir_kernel


def _patched_compile_bir_kernel(bir_json, tmpdir, neff_name="file.neff"):
    return _orig_compile_bir_kernel(_split_multiwait_bir(bir_json), tmpdir,
                                    neff_name=neff_name)


_bu.compile_bir_kernel = _patched_compile_bir_kernel
_b2j.compile_bir_kernel = _patched_compile_bir_kernel

# ---------------------------------------------------------------------------
# Problem constants (hardcoded per spec)
# ---------------------------------------------------------------------------
B, S, D, L, H = 2, 2048, 2048, 3, 16
DH = D // H            # 128
BLOCK = 16
NA = 64
Q = NA * BLOCK         # 1024
KV = S + Q             # 3072
VOCAB = 32000
MASK_ID = 31999
GAMMA = 7.0
EPS = 1e-6

LANES = 4
T = Q // LANES         # 256 draft tokens per core
C = S // LANES         # 512 ctx rows per core
E = Q // LANES         # 256 draft kv rows per core
CH = C + E             # 768 kv rows per core
LD = L * D             # 6144
KS = D // 128          # 16
FS = LD // 128         # 48
VT = 500               # vocab tile
NVT = VOCAB // VT      # 64
NJ = KV // 128         # 24 kv slices
HALF = DH // 2         # 64

BF = mybir.dt.bfloat16
F32 = mybir.dt.float32
MUL = mybir.AluOpType.mult
ADD = mybir.AluOpType.add
SUB = mybir.AluOpType.subtract
EXP = mybir.ActivationFunctionType.Exp

_BUILT = None


def _build():
    nc = bass.Bass()

    def din(name, shape, dt=BF):
        return nc.dram_tensor(name, shape, dt, kind="ExternalInput")

    wpT = din("wpT", [LD, D])
    catT = din("catT", [LD, C])
    wqT = din("wqT", [D, D])
    wkT = din("wkT", [D, D])
    wvT = din("wvT", [D, D])
    woT = din("woT", [D, D])
    embT = din("embT", [D, T])
    WhT = din("WhT", [D, VOCAB])
    cosq = din("cosq", [HALF, T], F32)
    sinq = din("sinq", [HALF, T], F32)
    cosk = din("cosk", [HALF, CH], F32)
    sink = din("sink", [HALF, CH], F32)
    maskT = din("maskT", [KV, T])
    ones1 = din("ones1", [128, 1])

    logits = nc.dram_tensor("logits", [T, VOCAB], F32, kind="ExternalOutput")

    kT_ch = nc.dram_tensor("kT_ch", [D, CH], BF)
    v_ch = nc.dram_tensor("v_ch", [CH, D], BF)
    kT_all = nc.dram_tensor("kT_all", [LANES * D, CH], BF)
    v_all = nc.dram_tensor("v_all", [LANES * CH, D], BF)
    srms = nc.dram_tensor("srms", [T, 1], F32)
    rcpd = nc.dram_tensor("rcpd", [H, T], F32)

    RG = [[0, 1, 2, 3], [4, 5, 6, 7]]

    with TileContext(nc) as tc:
        with tc.tile_pool(name="big", bufs=1) as big, \
             tc.tile_pool(name="w", bufs=4) as wp, \
             tc.tile_pool(name="tmp", bufs=4) as tp, \
             tc.tile_pool(name="mm", bufs=4, space="PSUM") as mmp, \
             tc.tile_pool(name="acc", bufs=2, space="PSUM") as accp, \
             tc.tile_pool(name="dn", bufs=2, space="PSUM") as dnp:

            # persistent SBUF
            cat_sb = big.tile([128, FS, C], BF)
            xkvT = big.tile([128, KS, CH], BF)
            qT_sb = big.tile([128, KS, T], BF)
            attnN = big.tile([128, H, T], BF)
            hT = big.tile([128, KS, T], BF)
            msk = big.tile([128, NJ, T], BF)
            ck = big.tile([HALF, CH], F32)
            sk = big.tile([HALF, CH], F32)
            cq = big.tile([HALF, T], F32)
            sq = big.tile([HALF, T], F32)
            on1 = big.tile([128, 1], BF)

            nc.sync.dma_start(ck[:], cosk[:])
            nc.sync.dma_start(sk[:], sink[:])
            nc.sync.dma_start(cq[:], cosq[:])
            nc.sync.dma_start(sq[:], sinq[:])
            nc.sync.dma_start(on1[:], ones1[:])
            nc.sync.dma_start(
                cat_sb[:], catT.rearrange("(fs p) s -> p fs s", p=128))
            nc.sync.dma_start(
                xkvT[:, :, C:CH], embT.rearrange("(ks p) t -> p ks t", p=128))
            for j in range(NJ):
                nc.sync.dma_start(msk[:, j], maskT[128 * j:128 * j + 128, :])

            # --- A: ctxT[d, s] into xkvT[:, :, 0:C] ---
            for m in range(KS):
                ps = mmp.tile([128, 512], F32, tag="mm")
                for f in range(FS):
                    wt = wp.tile([128, 128], BF, tag="w")
                    nc.sync.dma_start(
                        wt[:], wpT[128 * f:128 * f + 128,
                                   128 * m:128 * m + 128])
                    nc.tensor.matmul(ps[:], wt[:], cat_sb[:, f],
                                     start=(f == 0), stop=(f == FS - 1))
                nc.vector.tensor_copy(xkvT[:, m, 0:C], ps[:])

            def rope(psum, cos, sin, out_sb, n):
                # out[0:64]   = p[0:64]*c - p[64:128]*s
                # out[64:128] = p[64:128]*c + p[0:64]*s
                t1 = tp.tile([HALF, n], F32, tag="r1")
                t2 = tp.tile([HALF, n], F32, tag="r2")
                nc.vector.tensor_tensor(t1[:], psum[0:HALF], cos, MUL)
                nc.vector.tensor_tensor(t2[:], psum[HALF:128], sin, MUL)
                nc.vector.tensor_tensor(out_sb[0:HALF], t1[:], t2[:], SUB)
                t3 = tp.tile([HALF, n], F32, tag="r1")
                t4 = tp.tile([HALF, n], F32, tag="r2")
                nc.vector.tensor_tensor(t3[:], psum[HALF:128], cos, MUL)
                nc.vector.tensor_tensor(t4[:], psum[0:HALF], sin, MUL)
                nc.vector.tensor_tensor(out_sb[HALF:128], t3[:], t4[:], ADD)

            # --- B: kT chunk (rope'd) -> kT_ch ---
            NS = 384
            for m in range(KS):
                for ns in range(CH // NS):
                    ps = mmp.tile([128, 512], F32, tag="mm")
                    for k in range(KS):
                        wt = wp.tile([128, 128], BF, tag="w")
                        nc.sync.dma_start(
                            wt[:], wkT[128 * k:128 * k + 128,
                                       128 * m:128 * m + 128])
                        nc.tensor.matmul(
                            ps[:, 0:NS], wt[:],
                            xkvT[:, k, NS * ns:NS * ns + NS],
                            start=(k == 0), stop=(k == KS - 1))
                    kt = tp.tile([128, NS], BF, tag="kt")
                    rope(ps[:, 0:NS], ck[:, NS * ns:NS * ns + NS],
                         sk[:, NS * ns:NS * ns + NS], kt[:], NS)
                    nc.sync.dma_start(
                        kT_ch[128 * m:128 * m + 128, NS * ns:NS * ns + NS],
                        kt[:])

            nc.gpsimd.collective_compute(
                "AllGather", mybir.AluOpType.bypass,
                ins=[kT_ch[:]], outs=[kT_all[:]], replica_groups=RG)

            # --- C: v chunk -> v_ch ---
            for mv in range(CH // 128):
                for n in range(D // 512):
                    ps = mmp.tile([128, 512], F32, tag="mm")
                    for k in range(KS):
                        wt = wp.tile([128, 512], BF, tag="wv")
                        nc.sync.dma_start(
                            wt[:], wvT[128 * k:128 * k + 128,
                                       512 * n:512 * n + 512])
                        nc.tensor.matmul(
                            ps[:], xkvT[:, k, 128 * mv:128 * mv + 128], wt[:],
                            start=(k == 0), stop=(k == KS - 1))
                    vt = tp.tile([128, 512], BF, tag="vt")
                    nc.vector.tensor_copy(vt[:], ps[:])
                    nc.sync.dma_start(
                        v_ch[128 * mv:128 * mv + 128, 512 * n:512 * n + 512],
                        vt[:])

            nc.gpsimd.collective_compute(
                "AllGather", mybir.AluOpType.bypass,
                ins=[v_ch[:]], outs=[v_all[:]], replica_groups=RG)

            # --- E: qT (rope'd, scale folded into wqT) ---
            for m in range(KS):
                ps = mmp.tile([128, 512], F32, tag="mm")
                for k in range(KS):
                    wt = wp.tile([128, 128], BF, tag="w")
                    nc.sync.dma_start(
                        wt[:], wqT[128 * k:128 * k + 128,
                                   128 * m:128 * m + 128])
                    nc.tensor.matmul(ps[:, 0:T], wt[:], xkvT[:, k, C:CH],
                                     start=(k == 0), stop=(k == KS - 1))
                qt = tp.tile([128, T], BF, tag="qt")
                rope(ps[:, 0:T], cq[:], sq[:], qt[:], T)
                nc.vector.tensor_copy(qT_sb[:, m], qt[:])

            # --- F: attention (scoresT orientation, exp w/o max, mult mask)
            for h in range(H):
                dn = dnp.tile([1, T], F32, tag="dn")
                ov = accp.tile([128, T], F32, tag="ov")
                for j in range(NJ):
                    rr, u0 = j // (CH // 128), 128 * (j % (CH // 128))
                    kt = wp.tile([128, 128], BF, tag="kv")
                    nc.sync.dma_start(
                        kt[:], kT_all[D * rr + 128 * h:D * rr + 128 * h + 128,
                                      u0:u0 + 128])
                    ps = mmp.tile([128, 512], F32, tag="mm")
                    nc.tensor.matmul(ps[:, 0:T], kt[:], qT_sb[:, h],
                                     start=True, stop=True)
                    pe = tp.tile([128, T], F32, tag="pe")
                    nc.scalar.activation(pe[:], ps[:, 0:T], EXP)
                    pb = tp.tile([128, T], BF, tag="pb")
                    nc.vector.tensor_tensor(pb[:], pe[:], msk[:, j], MUL)
                    nc.tensor.matmul(dn[:], on1[:], pb[:],
                                     start=(j == 0), stop=(j == NJ - 1))
                    vt = wp.tile([128, 128], BF, tag="kv")
                    nc.sync.dma_start(
                        vt[:], v_all[128 * j:128 * j + 128,
                                     128 * h:128 * h + 128])
                    nc.tensor.matmul(ov[:], vt[:], pb[:],
                                     start=(j == 0), stop=(j == NJ - 1))
                rc = tp.tile([1, T], F32, tag="rc")
                nc.vector.reciprocal(rc[:], dn[:])
                nc.sync.dma_start(rcpd[h:h + 1, :], rc[:])
                rb = tp.tile([128, T], F32, tag="rb")
                nc.sync.dma_start(rb[:], rcpd[h:h + 1, :].to_broadcast((128, T)))
                nc.vector.tensor_tensor(attnN[:, h], ov[:], rb[:], MUL)

            # --- H: output projection + residual ---
            for m in range(KS):
                ps = mmp.tile([128, 512], F32, tag="mm")
                for k in range(KS):
                    wt = wp.tile([128, 128], BF, tag="w")
                    nc.sync.dma_start(
                        wt[:], woT[128 * k:128 * k + 128,
                                   128 * m:128 * m + 128])
                    nc.tensor.matmul(ps[:, 0:T], wt[:], attnN[:, k],
                                     start=(k == 0), stop=(k == KS - 1))
                nc.vector.tensor_tensor(hT[:, m], ps[:, 0:T],
                                        xkvT[:, m, C:CH], ADD)

            # --- I: RMS scale s_t = 1/sqrt(mean(h^2)+eps) ---
            msps = dnp.tile([1, T], F32, tag="dn")
            for m in range(KS):
                sqm = tp.tile([128, T], BF, tag="sq")
                nc.vector.tensor_tensor(sqm[:], hT[:, m], hT[:, m], MUL)
                nc.tensor.matmul(msps[:], on1[:], sqm[:],
                                 start=(m == 0), stop=(m == KS - 1))
            t1 = tp.tile([1, T], F32, tag="t1")
            nc.vector.tensor_scalar(t1[:], msps[:], 1.0 / D, EPS, MUL, ADD)
            t2 = tp.tile([1, T], F32, tag="t2")
            nc.scalar.sqrt(t2[:], t1[:])
            t3 = tp.tile([1, T], F32, tag="t3")
            nc.vector.reciprocal(t3[:], t2[:])
            nc.sync.dma_start(srms.rearrange("t one -> one t"), t3[:])
            sT0 = big.tile([128, 1], F32)
            sT1 = big.tile([128, 1], F32)
            nc.sync.dma_start(sT0[:], srms[0:128, :])
            nc.sync.dma_start(sT1[:], srms[128:256, :])

            # --- J: lm_head -> scaled logits out ---
            for qs in range(2):
                sT = sT0 if qs == 0 else sT1
                for v in range(NVT):
                    ps = mmp.tile([128, 512], F32, tag="mm")
                    for k in range(KS):
                        wt = wp.tile([128, VT], BF, tag="wh")
                        nc.sync.dma_start(
                            wt[:], WhT[128 * k:128 * k + 128,
                                       VT * v:VT * v + VT])
                        nc.tensor.matmul(
                            ps[:, 0:VT],
                            hT[:, k, 128 * qs:128 * qs + 128], wt[:],
                            start=(k == 0), stop=(k == KS - 1))
                    lg = tp.tile([128, VT], F32, tag="lg")
                    nc.vector.tensor_tensor(
                        lg[:], ps[:, 0:VT], sT[:].to_broadcast((128, VT)), MUL)
                    nc.sync.dma_start(
                        logits[128 * qs:128 * qs + 128, VT * v:VT * v + VT],
                        lg[:])

    return nc


def _host_prep(inputs):
    ids = np.asarray(inputs["input_ids"])
    hs = np.asarray(inputs["hidden_states"])
    anch = np.asarray(inputs["anchor_positions"])
    emb_tab = np.asarray(inputs["embed_table"])
    g = np.asarray(inputs["norm_weight"]).astype(np.float32)

    off = np.arange(Q, dtype=np.int64) % BLOCK
    anchor_per_tok = np.repeat(anch, BLOCK, axis=1)
    anchor_tok = np.take_along_axis(ids, anch, axis=1)
    draft_ids = np.where(off[None, :] == 0,
                         np.repeat(anchor_tok, BLOCK, axis=1), MASK_ID)
    labels = np.where(off[None, :] == 0, -100,
                      np.take_along_axis(ids, anchor_per_tok + off[None, :],
                                         axis=1))
    draft_pos = (anch[:, :, None]
                 + np.arange(BLOCK, dtype=anch.dtype)[None, None, :]
                 ).reshape(B, Q)
    emb = emb_tab[draft_ids]  # [B, Q, D] f32

    inv = 1.0 / (10000.0 ** (np.arange(HALF, dtype=np.float64) / HALF))

    def cs(pos):
        ang = pos.astype(np.float64)[:, None] * inv[None, :]
        return (np.cos(ang).T.astype(np.float32),
                np.sin(ang).T.astype(np.float32))

    b16 = ml_dtypes.bfloat16
    wpT = np.ascontiguousarray(np.asarray(inputs["ctx_proj_w"]).T).astype(b16)
    wqT = np.ascontiguousarray(
        (np.asarray(inputs["wq"]) / math.sqrt(DH)).T).astype(b16)
    wkT = np.ascontiguousarray(np.asarray(inputs["wk"]).T).astype(b16)
    wvT = np.ascontiguousarray(np.asarray(inputs["wv"]).T).astype(b16)
    woT = np.ascontiguousarray(np.asarray(inputs["wo"]).T).astype(b16)
    WhT = np.ascontiguousarray(
        (np.asarray(inputs["lm_head_weight"]) * g[None, :]).T).astype(b16)
    ones1 = np.ones((128, 1), b16)
    ctx_pos = np.arange(S, dtype=np.int64)

    in_maps = []
    for j in range(8):
        b, r = j // LANES, j % LANES
        catT = np.ascontiguousarray(
            hs[:, b, C * r:C * r + C, :].transpose(0, 2, 1).reshape(LD, C)
        ).astype(b16)
        embT = np.ascontiguousarray(emb[b, T * r:T * r + T, :].T).astype(b16)
        cq, sq = cs(draft_pos[b, T * r:T * r + T])
        posk = np.concatenate([ctx_pos[C * r:C * r + C],
                               draft_pos[b, T * r:T * r + T]])
        ckk, skk = cs(posk)
        gt = T * r + np.arange(T)
        qb = gt // BLOCK
        apos = anch[b, qb]                       # [T]
        maskT = np.zeros((KV, T), np.float32)
        for rr in range(LANES):
            s_idx = C * rr + np.arange(C)
            maskT[CH * rr:CH * rr + C, :] = (
                s_idx[:, None] < apos[None, :])
            gk = T * rr + np.arange(E)
            maskT[CH * rr + C:CH * rr + CH, :] = (
                (gk // BLOCK)[:, None] == qb[None, :])
        in_maps.append({
            "wpT": wpT, "catT": catT, "wqT": wqT, "wkT": wkT,
            "wvT": wvT, "woT": woT, "embT": embT, "WhT": WhT,
            "cosq": cq, "sinq": sq, "cosk": ckk, "sink": skk,
            "maskT": maskT.astype(b16), "ones1": ones1,
        })
    meta = dict(labels=labels, off=off, emb=emb, draft_pos=draft_pos)
    return in_maps, meta


def kernel(**inputs):
    global _BUILT
    if _BUILT is None:
        _BUILT = _build()
    nc = _BUILT
    in_maps, meta = _host_prep(inputs)
    res = None
    for attempt in range(3):
        try:
            res = run_bass_kernel_spmd(nc, in_maps, core_ids=list(range(8)))
            break
        except Exception:
            if attempt == 2:
                raise
            import time as _time
            _time.sleep(10.0)
    globals()["LAST_RESULT"] = res
    logits = np.zeros((B, Q, VOCAB), np.float32)
    for j in range(8):
        b, r = j // LANES, j % LANES
        logits[b, T * r:T * r + T] = res.results[j]["logits"]

    labels = meta["labels"]
    off = meta["off"]
    lab = np.where(labels < 0, 0, labels)
    m = logits.max(axis=-1)
    lse = m + np.log(np.exp(logits - m[..., None]).sum(axis=-1))
    lab_logit = np.take_along_axis(logits, lab[..., None], axis=-1)[..., 0]
    nll = lse - lab_logit
    valid = labels != -100
    offf = off.astype(np.float64)
    decay = np.where(off == 0, 0.0, np.exp(-(offf - 1.0) / GAMMA))
    w = decay[None, :] * valid.astype(np.float64)
    loss = float((nll * w).sum() / max(w.sum(), 1e-6))
    preds = logits.argmax(axis=-1)
    acc = float(((preds == labels) & valid).sum() / max(valid.sum(), 1))
    return np.array([loss, acc], np.float32)
